# revision 29
# baseline (speedup 1.0000x reference)
"""Trainium2 Bass kernel for a 2-layer GAT (50k nodes, 1.6M+50k edges) on 8
NeuronCores.

Edges are partitioned by dst range (6250/core), dst-sorted, packed into 98
windows of 64 dsts. Per window, edge rows are fetched with bulk dma_gather
instructions (one per window-group instead of one indirect DMA per 128-edge
chunk), using 256-bf16-elem padded table rows. The per-edge attention weight
uses the softmax scale-invariance identity
    exp(lrelu(as+ad) - ad) = max(exp(as), exp(0.2*as)*exp(-0.8*ad))
so the tables store u=exp(as), v=exp(0.2*as) per src and g=exp(-0.8*ad) per
dst; the inner loop is just w = max(u, v*g) folded into the features, and a
PSUM-accumulating one-hot matmul aggregates numerators + denominators.
dma_gather indices are int16, so gathers from the 50176-row tables are split
into a low (<32768) and high call per group with per-window static quotas.
"""
import sys

for _p in ("/opt/trn_rl_repo",):
    if _p not in sys.path:
        sys.path.insert(0, _p)

import numpy as np
import ml_dtypes

import concourse.bass as bass
import concourse.bacc as bacc
import concourse.mybir as mybir
import concourse.tile as tile
from concourse.bass_utils import run_bass_kernel_spmd
from concourse.masks import make_identity

BF = mybir.dt.bfloat16
F32 = mybir.dt.float32
I16 = mybir.dt.int16

N = 50000
IN_CH = 128
HID = 32
H1 = 4
H2 = 2
OUT_CH = 3
NEG = 0.2

NCORES = 8
LOCAL = 6250
DW = 64
NWIN = 98
LPAD = NWIN * DW          # 6272
NPAD = NCORES * LPAD      # 50176
P = 128
SPLIT = 32768
RROW = 256                # t1g row elems (bf16): [h 128 | u 4 | v 4 | pad]
R1 = 136
R1A = 140                 # w1aug cols: [W1 | W1As | .2 W1As | -.8 W1Ad]
R2ROW = 128               # t2full/dtab row elems
R2 = 68                   # [h2 64 | u2 2 | v2 2]
R2A = 70
GB = 5                    # windows per gather group
LAST_NC = None            # most recently built program (for benching)


def _bf(a):
    return np.ascontiguousarray(np.asarray(a, np.float32)).astype(ml_dtypes.bfloat16)


def _blockdiag(a, heads, ch):
    m = np.zeros((heads * ch, heads), np.float32)
    for h in range(heads):
        m[h * ch:(h + 1) * ch, h] = np.asarray(a, np.float32)[h]
    return m


def _groups():
    gs, w0 = [], 0
    while w0 < NWIN:
        b = min(GB, NWIN - w0)
        gs.append((w0, b))
        w0 += b
    return gs


def _wrap16(arr):
    # [n] int -> [128, n/16] int16 tape block (i -> [i%16, i//16], tiled x8)
    n = len(arr)
    t = np.asarray(arr, np.int64).reshape(n // 16, 16).T.astype(np.int16)
    return np.tile(t, (8, 1))


def _build_tapes(win_edges, Flo, Fhi):
    """win_edges: per window list of (srcid, dstl) with srcid the table row.
    Returns int16 hs-idx tape [128, NWIN*F*8], int16 g-idx tape (same cols),
    bf16 dstl tape [128, NWIN*F]; chunk order per group: all lo chunks
    (window-major), then all hi chunks."""
    F = Flo + Fhi
    hs_cols, g_cols, dl_cols = [], [], []
    for (w0, b) in _groups():
        lo_ids = np.zeros((b, Flo * P), np.int64)
        hi_ids = np.zeros((b, Fhi * P), np.int64)
        lo_g = np.zeros((b, Flo * P), np.int64)
        hi_g = np.zeros((b, Fhi * P), np.int64)
        lo_dl = np.full((b, Flo * P), DW, np.int64)
        hi_dl = np.full((b, Fhi * P), DW, np.int64)
        for i in range(b):
            w = w0 + i
            ids, dls = win_edges[w]
            m = ids < SPLIT
            nlo = int(m.sum())
            nhi = len(ids) - nlo
            assert nlo <= Flo * P and nhi <= Fhi * P, (w, nlo, nhi)
            lo_ids[i, :nlo] = ids[m]
            lo_g[i, :nlo] = w * DW + dls[m]
            lo_dl[i, :nlo] = dls[m]
            hi_ids[i, :nhi] = ids[~m] - SPLIT
            hi_g[i, :nhi] = w * DW + dls[~m]
            hi_dl[i, :nhi] = dls[~m]
        hs_cols.append(_wrap16(lo_ids.reshape(-1)))
        hs_cols.append(_wrap16(hi_ids.reshape(-1)))
        g_cols.append(_wrap16(lo_g.reshape(-1)))
        g_cols.append(_wrap16(hi_g.reshape(-1)))
        dl_cols.append(lo_dl.reshape(b * Flo, P).T)
        dl_cols.append(hi_dl.reshape(b * Fhi, P).T)
    hs_t = np.concatenate(hs_cols, axis=1)
    g_t = np.concatenate(g_cols, axis=1)
    dl_t = np.concatenate(dl_cols, axis=1).astype(np.float32)
    return (np.ascontiguousarray(hs_t), np.ascontiguousarray(g_t),
            np.ascontiguousarray(_bf(dl_t)))


def _build_program(F1lo, F1hi, F2lo, F2hi):
    nc = bacc.Bacc("TRN2", target_bir_lowering=False, debug=False,
                   num_devices=NCORES)
    F1 = F1lo + F1hi
    F2 = F2lo + F2hi
    C1 = NWIN * F1
    C2 = NWIN * F2
    NT1 = NPAD // P   # 392
    NTD = LPAD // P   # 49

    xT = nc.dram_tensor("xT", [P, NPAD], BF, kind="ExternalInput")
    xdT = nc.dram_tensor("xdT", [P, LPAD], BF, kind="ExternalInput")
    w1aug = nc.dram_tensor("w1aug", [P, R1A], BF, kind="ExternalInput")
    w2aug = nc.dram_tensor("w2aug", [P, R2A], BF, kind="ExternalInput")
    wlin = nc.dram_tensor("wlin", [HID, OUT_CH], BF, kind="ExternalInput")
    blin = nc.dram_tensor("blin", [OUT_CH, 1], F32, kind="ExternalInput")
    b1r = nc.dram_tensor("b1r", [1, P], F32, kind="ExternalInput")
    b2r = nc.dram_tensor("b2r", [1, HID], F32, kind="ExternalInput")
    s1t = nc.dram_tensor("s1t", [P, C1 * 8], I16, kind="ExternalInput")
    g1t = nc.dram_tensor("g1t", [P, C1 * 8], I16, kind="ExternalInput")
    l1t = nc.dram_tensor("l1t", [P, C1], BF, kind="ExternalInput")
    s2t = nc.dram_tensor("s2t", [P, C2 * 8], I16, kind="ExternalInput")
    g2t = nc.dram_tensor("g2t", [P, C2 * 8], I16, kind="ExternalInput")
    l2t = nc.dram_tensor("l2t", [P, C2], BF, kind="ExternalInput")

    yt = nc.dram_tensor("yt", [OUT_CH, LPAD], F32, kind="ExternalOutput")

    t1g = nc.dram_tensor("t1g", [NPAD * RROW], BF)
    d1tab = nc.dram_tensor("d1tab", [LPAD * R2ROW], BF)
    # t2loc/t2pack split in two at a group-aligned window boundary so the
    # first AllGather's deps close before L1 finishes and the first repack
    # can overlap the second AllGather.
    WSPLIT = 50                      # windows 0-49 | 50-97
    HROWS = [(0, WSPLIT * DW), (WSPLIT * DW, LPAD - WSPLIT * DW)]
    t2loc_h = [nc.dram_tensor(f"t2loc{i}", [nr * R2], BF)
               for i, (_, nr) in enumerate(HROWS)]
    t2pack_h = [nc.dram_tensor(f"t2pack{i}", [NCORES * nr * R2], BF,
                               addr_space="Shared")
                for i, (_, nr) in enumerate(HROWS)]
    t2full = nc.dram_tensor("t2full", [NPAD * R2ROW], BF)
    d2tab = nc.dram_tensor("d2tab", [LPAD * R2ROW], BF)

    groups = _groups()

    with tile.TileContext(nc) as tc:
        with tc.tile_pool(name="sb", bufs=2) as sb, \
             tc.tile_pool(name="eb", bufs=2) as eb, \
             tc.tile_pool(name="cst", bufs=1) as cst, \
             tc.tile_pool(name="ps", bufs=2, space="PSUM") as ps, \
             tc.tile_pool(name="pss", bufs=2, space="PSUM") as pss:

            ident = cst.tile([P, P], BF)
            make_identity(nc, ident[:])
            iota_i = cst.tile([P, DW], mybir.dt.int32)
            nc.gpsimd.iota(iota_i[:], pattern=[[1, DW]], base=0,
                           channel_multiplier=0)
            iota_b = cst.tile([P, DW], BF)
            nc.vector.tensor_copy(out=iota_b[:], in_=iota_i[:])
            w1aug_t = cst.tile([P, R1A], BF)
            nc.sync.dma_start(out=w1aug_t[:], in_=w1aug[:])
            w2aug_t = cst.tile([P, R2A], BF)
            nc.sync.dma_start(out=w2aug_t[:], in_=w2aug[:])
            wlin_t = cst.tile([HID, OUT_CH], BF)
            nc.sync.dma_start(out=wlin_t[:], in_=wlin[:])
            blin_t = cst.tile([OUT_CH, 1], F32)
            nc.sync.dma_start(out=blin_t[:], in_=blin[:])
            ones1 = cst.tile([1, P], F32)
            nc.gpsimd.memset(ones1[:], 1.0)
            b1r_t = cst.tile([1, P], F32)
            nc.sync.dma_start(out=b1r_t[:], in_=b1r[:])
            b2r_t = cst.tile([1, HID], F32)
            nc.sync.dma_start(out=b2r_t[:], in_=b2r[:])
            brep_ps = ps.tile([P, P], F32, tag="sc")
            nc.tensor.matmul(out=brep_ps[:], lhsT=ones1[:], rhs=b1r_t[:],
                             start=True, stop=True)
            b1rep = cst.tile([P, P], F32)
            nc.vector.tensor_copy(out=b1rep[:], in_=brep_ps[:])
            brep2_ps = ps.tile([P, HID], F32, tag="sc")
            nc.tensor.matmul(out=brep2_ps[:], lhsT=ones1[:], rhs=b2r_t[:],
                             start=True, stop=True)
            b2rep = cst.tile([P, HID], F32)
            nc.vector.tensor_copy(out=b2rep[:], in_=brep2_ps[:])

            # stage A: feature tables. 3 tiles per PSUM batch.
            def stage_a(src_dram, ntiles, payload, write_fn):
                # 12-tile DMA batches; PSUM limits matmul batches to 3 tiles.
                BT = 12
                for t0 in range(0, ntiles, BT):
                    bt = min(BT, ntiles - t0)
                    xs = sb.tile([P, BT * P], BF, tag="xs")
                    nc.sync.dma_start(out=xs[:, 0:bt * P],
                                      in_=src_dram[:, t0 * P:(t0 + bt) * P])
                    hsb = sb.tile([P, BT, R1], BF, tag="hsb")
                    hvs = []
                    for s0 in range(0, bt, 3):
                        sn = min(3, bt - s0)
                        hps = ps.tile([P, 3 * R1A], F32, tag="hg")
                        for j in range(sn):
                            nc.tensor.matmul(
                                out=hps[:, j * R1A:(j + 1) * R1A],
                                lhsT=xs[:, (s0 + j) * P:(s0 + j + 1) * P],
                                rhs=w1aug_t[:], start=True, stop=True)
                        hv = hps[:].rearrange("p (b r) -> p b r", r=R1A)
                        nc.vector.tensor_copy(
                            out=hsb[:, s0:s0 + sn, 0:P], in_=hv[:, :sn, 0:P])
                        nc.scalar.activation(
                            out=hsb[:, s0:s0 + sn, P:R1], in_=hv[:, :sn, P:R1],
                            func=mybir.ActivationFunctionType.Exp)
                        hvs.append((s0, sn, hv))
                    write_fn(t0, bt, hsb, hvs)

            def wr_global(t0, bt, hsb, hvs):
                out_ap = t1g[t0 * P * RROW:(t0 + bt) * P * RROW] \
                    .rearrange("(b p r) -> p b r", p=P, r=RROW)[:, :, 0:R1]
                nc.sync.dma_start(out=out_ap, in_=hsb[:, :bt, :])

            def wr_local(t0, bt, hsb, hvs):
                # local dst table: g rows
                gtmp = sb.tile([P, 12, H1], BF, tag="gtmp")
                for s0, sn, hv in hvs:
                    nc.scalar.activation(out=gtmp[:, s0:s0 + sn, :],
                                         in_=hv[:, :sn, R1:R1A],
                                         func=mybir.ActivationFunctionType.Exp)
                dst_ap = d1tab[t0 * P * R2ROW:(t0 + bt) * P * R2ROW] \
                    .rearrange("(b p r) -> p b r", p=P, r=R2ROW)[:, :, 0:H1]
                nc.sync.dma_start(out=dst_ap, in_=gtmp[:, :bt, :])

            stage_a(xT, NT1, R1, wr_global)
            stage_a(xdT, NTD, R1, wr_local)

            def edge_layer(Flo, Fhi, stape, gtape, ltape, tab, tab_rows,
                           rowsz, dtab, heads, hwid, epilogue):
                F = Flo + Fhi
                rhw = hwid + heads
                tab2d = tab.rearrange("(r e) -> r e", e=rowsz)
                tab2d_hi = tab[SPLIT * rowsz:].rearrange("(r e) -> r e",
                                                         e=rowsz)
                dtab2d = dtab.rearrange("(r e) -> r e", e=R2ROW)
                ccol = 0   # running chunk offset (tape col units)
                for (w0, b) in groups:
                    nlo, nhi = b * Flo * P, b * Fhi * P
                    nch = b * F
                    sidx = sb.tile([P, nch * 8], I16, tag="sidx")
                    nc.sync.dma_start(
                        out=sidx[:], in_=stape[:, ccol * 8:(ccol + nch) * 8])
                    gidx = sb.tile([P, nch * 8], I16, tag="gidx")
                    nc.sync.dma_start(
                        out=gidx[:], in_=gtape[:, ccol * 8:(ccol + nch) * 8])
                    dstl = sb.tile([P, nch], BF, tag="dstl")
                    nc.sync.dma_start(out=dstl[:],
                                      in_=ltape[:, ccol:ccol + nch])

                    # SWDGE ring holds ~1024 descriptors; cap each gather call
                    CAP = 1024

                    def gather_capped(dst, src2d, idxt, i0, n, elem):
                        for o in range(0, n, CAP):
                            nn = min(CAP, n - o)
                            c0 = (o // P)
                            nc.gpsimd.dma_gather(
                                out_ap=dst[:, c0:c0 + nn // P, :],
                                in_ap=src2d,
                                idxs_ap=idxt[:, (i0 + o) // 16:
                                             (i0 + o + nn) // 16],
                                num_idxs=nn, num_idxs_reg=nn, elem_size=elem)

                    hs = sb.tile([P, nch, rowsz], BF, tag="hs")
                    gather_capped(hs[:, 0:b * Flo, :], tab2d[0:SPLIT, :],
                                  sidx, 0, nlo, rowsz)
                    gather_capped(hs[:, b * Flo:nch, :],
                                  tab2d_hi[0:tab_rows - SPLIT, :],
                                  sidx, nlo, nhi, rowsz)
                    gt = sb.tile([P, nch, R2ROW], BF, tag="gt")
                    gather_capped(gt[:, :, :], dtab2d[:, :], gidx, 0, nch * P,
                                  R2ROW)

                    # one-hot [P, nch, DW]
                    s_sb = sb.tile([P, nch, DW], BF, tag="s_sb")
                    nc.vector.tensor_tensor(
                        out=s_sb[:],
                        in0=iota_b[:][:, None, :].to_broadcast([P, nch, DW]),
                        in1=dstl[:][:, :, None].to_broadcast([P, nch, DW]),
                        op=mybir.AluOpType.is_equal)
                    # w = max(u, v*g)
                    wt = sb.tile([P, nch, 4], BF, tag="wt")
                    nc.vector.tensor_tensor(
                        out=wt[:, :, 0:heads],
                        in0=hs[:, :, hwid + heads:hwid + 2 * heads],
                        in1=gt[:, :, 0:heads], op=mybir.AluOpType.mult)
                    # in-place: w = max(u, v*g) lands directly in the rhs
                    # denominator columns (hs[:, :, hwid:hwid+heads])
                    nc.vector.tensor_tensor(
                        out=hs[:, :, hwid:hwid + heads],
                        in0=wt[:, :, 0:heads],
                        in1=hs[:, :, hwid:hwid + heads],
                        op=mybir.AluOpType.max)
                    # fold w into features (per head); w already sits in
                    # the denominator columns
                    fh = hwid // heads
                    for h in range(heads):
                        nc.vector.tensor_tensor(
                            out=hs[:, :, h * fh:(h + 1) * fh],
                            in0=hs[:, :, h * fh:(h + 1) * fh],
                            in1=hs[:, :, hwid + h:hwid + h + 1]
                                .to_broadcast([P, nch, fh]),
                            op=mybir.AluOpType.mult)

                    for i in range(b):
                        agg = pss.tile([DW, rhw], F32, tag="agg")
                        chunks = ([i * Flo + j for j in range(Flo)] +
                                  [b * Flo + i * Fhi + j for j in range(Fhi)])
                        for k, c in enumerate(chunks):
                            nc.tensor.matmul(out=agg[:], lhsT=s_sb[:, c, :],
                                             rhs=hs[:, c, 0:rhw],
                                             start=(k == 0),
                                             stop=(k == len(chunks) - 1))
                        epilogue(w0 + i, w0, b, i, agg)
                    ccol += nch

            # epilogue staging tiles (per group)
            stage = {}

            def epi1(w, w0, b, i, agg):
                if i == 0:
                    stage["h2"] = eb.tile([DW, GB, R2], BF, tag="st2", name="st2h")
                    stage["g2"] = eb.tile([DW, GB, H2], BF, tag="stg", name="stgg")
                rec = sb.tile([DW, H1], F32, tag="rec")
                nc.vector.reciprocal(out=rec[:], in_=agg[:, P:P + H1])
                v = sb.tile([DW, P], F32, tag="v")
                nc.vector.tensor_tensor(
                    out=v[:].rearrange("d (h f) -> d h f", h=H1),
                    in0=agg[:, 0:P].rearrange("d (h f) -> d h f", h=H1),
                    in1=rec[:][:, :, None].to_broadcast([DW, H1, HID]),
                    op=mybir.AluOpType.mult)
                nc.vector.tensor_tensor(out=v[:], in0=v[:], in1=b1rep[:DW, :],
                                        op=mybir.AluOpType.add)
                m = sb.tile([DW, P], F32, tag="m")
                nc.vector.tensor_scalar_min(m[:], v[:], 0.0)
                nc.scalar.activation(out=m[:], in_=m[:],
                                     func=mybir.ActivationFunctionType.Exp)
                nc.vector.tensor_scalar_sub(m[:], m[:], 1.0)
                x2 = sb.tile([DW, P], BF, tag="x2")
                nc.vector.tensor_tensor(out=x2[:], in0=v[:], in1=m[:],
                                        op=mybir.AluOpType.max)
                x2T_ps = ps.tile([P, DW], BF, tag="sc")
                nc.tensor.transpose(out=x2T_ps[:], in_=x2[:],
                                    identity=ident[:DW, :DW])
                x2T = sb.tile([P, DW], BF, tag="x2T")
                nc.scalar.copy(out=x2T[:], in_=x2T_ps[:])
                h2_ps = ps.tile([DW, R2A], F32, tag="hg2")
                nc.tensor.matmul(out=h2_ps[:], lhsT=x2T[:], rhs=w2aug_t[:],
                                 start=True, stop=True)
                nc.vector.tensor_copy(out=stage["h2"][:, i, 0:HID * H2],
                                      in_=h2_ps[:, 0:HID * H2])
                nc.scalar.activation(out=stage["h2"][:, i, HID * H2:R2],
                                     in_=h2_ps[:, HID * H2:HID * H2 + 2 * H2],
                                     func=mybir.ActivationFunctionType.Exp)
                nc.scalar.activation(out=stage["g2"][:, i, :],
                                     in_=h2_ps[:, R2A - H2:R2A],
                                     func=mybir.ActivationFunctionType.Exp)
                if i == b - 1:
                    hx = 0 if w0 < WSPLIT else 1
                    r0 = (w0 - (0 if hx == 0 else WSPLIT)) * DW
                    out_ap = t2loc_h[hx][r0 * R2:(r0 + b * DW) * R2] \
                        .rearrange("(b d e) -> d b e", d=DW, e=R2)
                    nc.sync.dma_start(out=out_ap, in_=stage["h2"][:, 0:b, :])
                    dst_ap = d2tab[w0 * DW * R2ROW:(w0 + b) * DW * R2ROW] \
                        .rearrange("(b d e) -> d b e", d=DW, e=R2ROW)[:, :, 0:H2]
                    nc.sync.dma_start(out=dst_ap, in_=stage["g2"][:, 0:b, :])

            def epi2(w, w0, b, i, agg):
                if i == 0:
                    stage["y"] = eb.tile([OUT_CH, GB, DW], F32, tag="sty", name="styy")
                rec = sb.tile([DW, H2], F32, tag="rec")
                nc.vector.reciprocal(out=rec[:], in_=agg[:, 64:64 + H2])
                v = sb.tile([DW, 64], F32, tag="v2")
                nc.vector.tensor_tensor(
                    out=v[:].rearrange("d (h f) -> d h f", h=H2),
                    in0=agg[:, 0:64].rearrange("d (h f) -> d h f", h=H2),
                    in1=rec[:][:, :, None].to_broadcast([DW, H2, HID]),
                    op=mybir.AluOpType.mult)
                x3 = sb.tile([DW, HID], F32, tag="x3f")
                nc.vector.tensor_tensor(out=x3[:], in0=v[:, 0:HID],
                                        in1=v[:, HID:64],
                                        op=mybir.AluOpType.add)
                nc.scalar.mul(out=x3[:], in_=x3[:], mul=0.5)
                nc.vector.tensor_tensor(out=x3[:], in0=x3[:],
                                        in1=b2rep[:DW, :],
                                        op=mybir.AluOpType.add)
                m = sb.tile([DW, HID], F32, tag="m2")
                nc.vector.tensor_scalar_min(m[:], x3[:], 0.0)
                nc.scalar.activation(out=m[:], in_=m[:],
                                     func=mybir.ActivationFunctionType.Exp)
                nc.vector.tensor_scalar_sub(m[:], m[:], 1.0)
                x3b = sb.tile([DW, HID], BF, tag="x3b")
                nc.vector.tensor_tensor(out=x3b[:], in0=x3[:], in1=m[:],
                                        op=mybir.AluOpType.max)
                x3T_ps = ps.tile([HID, DW], BF, tag="sc")
                nc.tensor.transpose(out=x3T_ps[:], in_=x3b[:],
                                    identity=ident[:DW, :DW])
                x3T = sb.tile([HID, DW], BF, tag="x3T")
                nc.scalar.copy(out=x3T[:], in_=x3T_ps[:])
                y_ps = ps.tile([OUT_CH, DW], F32, tag="hg2")
                nc.tensor.matmul(out=y_ps[:], lhsT=wlin_t[:], rhs=x3T[:],
                                 start=True, stop=True)
                nc.vector.tensor_scalar(out=stage["y"][:, i, :], in0=y_ps[:],
                                        scalar1=blin_t[:, :1], scalar2=None,
                                        op0=mybir.AluOpType.add)
                if i == b - 1:
                    nc.sync.dma_start(
                        out=yt[:, w0 * DW:(w0 + b) * DW],
                        in_=stage["y"][:, 0:b, :].rearrange(
                            "o b d -> o (b d)"))

            edge_layer(F1lo, F1hi, s1t, g1t, l1t, t1g, NPAD, RROW, d1tab,
                       H1, P, epi1)

            # AllGather each half as soon as its t2loc writes complete;
            # repack half 0 overlaps the AllGather of half 1.
            RPT = 24
            rp_out = t2full.rearrange("(r e) -> r e", e=R2ROW)[:, 0:R2]
            for hx, (r0h, nrh) in enumerate(HROWS):
                nc.gpsimd.collective_compute(
                    "AllGather", mybir.AluOpType.bypass,
                    replica_groups=[list(range(NCORES))],
                    ins=[t2loc_h[hx][:]], outs=[t2pack_h[hx][:]])
                for c in range(NCORES):
                    base = c * nrh * R2
                    rows0 = c * LPAD + r0h
                    rp_in = t2pack_h[hx][base:base + nrh * R2] \
                        .rearrange("(r e) -> r e", e=R2)
                    for b0 in range(0, nrh // P, RPT):
                        bfull = min(RPT, nrh // P - b0)
                        r0 = b0 * P
                        st = sb.tile([P, RPT, R2], BF, tag="rp")
                        nc.sync.dma_start(
                            out=st[:, 0:bfull, :],
                            in_=rp_in[r0:r0 + bfull * P, :]
                                .rearrange("(b p) e -> p b e", p=P))
                        nc.sync.dma_start(
                            out=rp_out[rows0 + r0:rows0 + r0 + bfull * P, :]
                                .rearrange("(b p) e -> p b e", p=P),
                            in_=st[:, 0:bfull, :])

            edge_layer(F2lo, F2hi, s2t, g2t, l2t, t2full, NPAD, R2ROW,
                       d2tab, H2, 64, epi2)

    nc.compile()
    return nc


def kernel(x, edge_index, W1, a_src1, a_dst1, b1, W2, a_src2, a_dst2, b2,
           W_lin, b_lin):
    x = np.asarray(x, np.float32)
    edge_index = np.asarray(edge_index)

    xpad = np.zeros((NPAD, IN_CH), np.float32)
    xpad[:N] = x
    xT_bf = np.ascontiguousarray(_bf(xpad).T)

    W1f = np.asarray(W1, np.float32)
    As1 = W1f @ _blockdiag(a_src1, H1, HID)
    Ad1 = W1f @ _blockdiag(a_dst1, H1, HID)
    w1aug_bf = _bf(np.concatenate([W1f, As1, NEG * As1, -0.8 * Ad1], axis=1))
    W2f = np.asarray(W2, np.float32)
    As2 = W2f @ _blockdiag(a_src2, H2, HID)
    Ad2 = W2f @ _blockdiag(a_dst2, H2, HID)
    w2aug_bf = _bf(np.concatenate([W2f, As2, NEG * As2, -0.8 * Ad2], axis=1))
    wlin_bf = _bf(W_lin)
    blin_col = np.asarray(b_lin, np.float32).reshape(OUT_CH, 1)
    b1_row = np.asarray(b1, np.float32).reshape(1, P)
    b2_row = np.asarray(b2, np.float32).reshape(1, HID)

    src = np.concatenate([edge_index[0].astype(np.int64),
                          np.arange(N, dtype=np.int64)])
    dst = np.concatenate([edge_index[1].astype(np.int64),
                          np.arange(N, dtype=np.int64)])

    # Degree-balanced window packing (snake round-robin of degree-sorted dsts)
    pc = []
    newlocals = []
    for k in range(NCORES):
        m = (dst >= k * LOCAL) & (dst < (k + 1) * LOCAL)
        idx = np.nonzero(m)[0]
        dl_old = (dst[idx] - k * LOCAL).astype(np.int64)
        deg = np.bincount(dl_old, minlength=LOCAL)
        order = np.argsort(-deg, kind="stable")
        i = np.arange(LOCAL)
        blk = i // NWIN
        win = np.where(blk % 2 == 0, i % NWIN, NWIN - 1 - (i % NWIN))
        newlocal = np.empty(LOCAL, np.int64)
        newlocal[order] = win * DW + blk
        newlocals.append(newlocal)
        pc.append((idx, newlocal[dl_old]))

    remap = np.zeros(N, np.int64)
    for k in range(NCORES):
        remap[k * LOCAL:(k + 1) * LOCAL] = LPAD * k + newlocals[k]

    # per-core, per-window edge lists; compute global quotas
    def win_split(ids_dl):
        ids, dl = ids_dl
        w = dl // DW
        out = []
        for wi in range(NWIN):
            m = w == wi
            out.append((ids[m], dl[m] % DW))
        return out

    per_core_w1, per_core_w2 = [], []
    F1lo = F1hi = F2lo = F2hi = 1
    for k in range(NCORES):
        idx, dl = pc[k]
        s1 = src[idx]            # original node id = t1g row
        s2 = remap[src[idx]]     # remapped id = t2full row
        wl1 = win_split((s1, dl))
        wl2 = win_split((s2, dl))
        per_core_w1.append(wl1)
        per_core_w2.append(wl2)
        for wl, att in ((wl1, "1"), (wl2, "2")):
            for ids, _ in wl:
                nlo = int((ids < SPLIT).sum())
                nhi = len(ids) - nlo
                if att == "1":
                    F1lo = max(F1lo, (nlo + P - 1) // P)
                    F1hi = max(F1hi, (nhi + P - 1) // P)
                else:
                    F2lo = max(F2lo, (nlo + P - 1) // P)
                    F2hi = max(F2hi, (nhi + P - 1) // P)

    nc = _build_program(F1lo, F1hi, F2lo, F2hi)
    global LAST_NC
    LAST_NC = nc

    in_maps = []
    for k in range(NCORES):
        s1, g1, l1 = _build_tapes(per_core_w1[k], F1lo, F1hi)
        s2, g2, l2 = _build_tapes(per_core_w2[k], F2lo, F2hi)
        xd = np.zeros((LPAD, IN_CH), np.float32)
        xd[newlocals[k]] = x[k * LOCAL:(k + 1) * LOCAL]
        xdT_bf = np.ascontiguousarray(_bf(xd).T)
        in_maps.append({
            "xT": xT_bf, "xdT": xdT_bf, "w1aug": w1aug_bf, "w2aug": w2aug_bf,
            "wlin": wlin_bf, "blin": blin_col, "b1r": b1_row, "b2r": b2_row,
            "s1t": s1, "g1t": g1, "l1t": l1,
            "s2t": s2, "g2t": g2, "l2t": l2,
        })

    res = run_bass_kernel_spmd(nc, in_maps, list(range(NCORES)))
    out = np.zeros((N, OUT_CH), np.float32)
    for k in range(NCORES):
        ytk = res.results[k]["yt"]
        out[k * LOCAL:(k + 1) * LOCAL] = ytk[:, newlocals[k]].T
    return out
